# revision 11
# baseline (speedup 1.0000x reference)
"""Bass/Trainium2 kernel for the GCL loss function (nn_GCL_46076409151702).

Math (reference):
    g_s = segment_sum(z_s, batch_s, 512)            s in {1,2}
    zn_s, gn_s = l2norm rows
    pos11 = <zn1[i], gn1[b1[i]]>, cross12 = <zn1[i], gn2[b1[i]]>
    pos22 = <zn2[i], gn2[b2[i]]>, cross21 = <zn2[i], gn1[b2[i]]>
    d_s = softplus(-cross) - softplus(-pos)
    out  = sqrt(sum d1^2) + sqrt(sum d2^2)

Sharding: batch ids are SORTED, so splitting the 512 graphs into 8
contiguous blocks of 64 gives each core a contiguous row range whose
segment sums are fully local -- NO collective at all.  Each core handles
graphs [64c, 64c+64); its row shard (per side) is exactly the rows whose
batch falls in that window, zero-padded to a fixed NT*128 rows.

z ships in fp8 e4m3 (error ~1e-3 on the final loss, gate is 2e-2) in two
host-prepared layouts, both contiguous per partition: tile-interleaved
natural zn[p, t*128+d] = z[t*128+p, d] for the segment matmuls, and
transposed zt[d, i] = z[i, d] for the dot matvecs.  Everything overlaps
the chunked DMA stream:

  P1  per 128-node tile: seg-matmul (lhsT=z tile, rhs=2-col fp8 mask)
  S   row norms: squares on Scalar (side 0) / DVE (side 1), row sums via
      per-tile ones-matmuls on Tensor (sq tile as weights)
  P2  transpose stag, scatter into local g[64,128] via one-hot matmuls
  P3  normalize g rows (per-partition scale), cast bf16
  P5  gather candidate columns via one-hot gather-matmuls, cast fp8
  P6  per tile: matvec (lhsT=zT fp8 tile, rhs=4 candidate cols)
  P7  select by mask (folded with 1/||z||), single batched Softplus,
      accumulate d^2; per-core partial [2] -> host: sqrt + sqrt
"""

import numpy as np
import ml_dtypes

import concourse.bass as bass
import concourse.bacc as bacc
import concourse.mybir as mybir
import concourse.tile as tile
from concourse.bass_utils import run_bass_kernel_spmd
from concourse.masks import make_identity

F32 = mybir.dt.float32
BF16 = mybir.dt.bfloat16
FP8 = mybir.dt.float8e4
AL = mybir.AluOpType
AF = mybir.ActivationFunctionType

NCORES = 8
G = 512          # num graphs
GPC = G // NCORES  # graphs per core = 64
D = 128          # feature dim
P = 128          # partitions
NT = 103         # tiles per core (12500 expected rows + slack)
R = NT * P       # rows per core = 13184
CK = 2           # candidate graphs per tile (sorted batch)
NIDX = NT * 2 * CK * 2   # 824 gather cols
NIDXP = 832      # padded
EPS = 1e-12
HT = 52          # tiles per P6 psum bank
CHN = [17, 17, 17, 17, 17, 18]   # z natural chunk sizes (tiles)
CHT = [34, 34, 35]               # zT chunk sizes (tiles)


def build_nc(finalize=True):
    nc = bacc.Bacc(None, target_bir_lowering=False, debug=False)
    # tile-interleaved natural layout: zn[p, t*128+d] = z[t*128+p, d]
    z1 = nc.dram_tensor("z1", [P, R], BF16, kind="ExternalInput")
    z2 = nc.dram_tensor("z2", [P, R], BF16, kind="ExternalInput")
    zt1 = nc.dram_tensor("zt1", [P, R], FP8, kind="ExternalInput")
    zt2 = nc.dram_tensor("zt2", [P, R], FP8, kind="ExternalInput")
    # interleaved per-tile masks: mab[p, CK*t+j] = (batch[t*128+p] == A_t + j)
    mab1 = nc.dram_tensor("mab1", [P, NT * CK], FP8, kind="ExternalInput")
    mab2 = nc.dram_tensor("mab2", [P, NT * CK], FP8, kind="ExternalInput")
    mabb1 = nc.dram_tensor("mabb1", [P, NT * CK], BF16, kind="ExternalInput")
    mabb2 = nc.dram_tensor("mabb2", [P, NT * CK], BF16, kind="ExternalInput")
    # scatter one-hot: sel[c, w] = 1 iff a_{c//CK} + c%CK == w (local window)
    sel1 = nc.dram_tensor("sel1", [2 * P, GPC], F32, kind="ExternalInput")
    sel2 = nc.dram_tensor("sel2", [2 * P, GPC], F32, kind="ExternalInput")
    # gather one-hot: e_s[w, j] = 1 iff candidate col j sources side s graph w
    e1 = nc.dram_tensor("e1", [GPC, NIDXP], BF16, kind="ExternalInput")
    e2 = nc.dram_tensor("e2", [GPC, NIDXP], BF16, kind="ExternalInput")
    out_part = nc.dram_tensor("out_part", [2, 1], F32, kind="ExternalOutput")

    zs = [z1, z2]
    zts = [zt1, zt2]
    mabs = [mab1, mab2]
    mabbs = [mabb1, mabb2]
    sels = [sel1, sel2]
    es = [e1, e2]

    def _body(tc):
        with (
            tc.tile_pool(name="const", bufs=1) as constp,
            tc.tile_pool(name="stash", bufs=1) as stashp,
            tc.tile_pool(name="zin", bufs=3) as zinp,
            tc.tile_pool(name="sq", bufs=2) as sqp,
            tc.tile_pool(name="small", bufs=4) as smallp,
            tc.tile_pool(name="pstag", bufs=1, space="PSUM") as pstagp,
            tc.tile_pool(name="pcand", bufs=1, space="PSUM") as pcandp,
            tc.tile_pool(name="pg", bufs=1, space="PSUM") as pgp,
            tc.tile_pool(name="pssq", bufs=1, space="PSUM") as pssqp,
            tc.tile_pool(name="pzt", bufs=2, space="PSUM") as pztp,
        ):
            # ---- constants / small inputs to SBUF (sync queue) ----
            ident = constp.tile([P, P], F32)
            make_identity(nc, ident[:])
            ones_col = constp.tile([P, 1], F32)
            nc.vector.memset(ones_col[:], 1.0)
            ones_bf = constp.tile([P, 1], BF16)
            nc.vector.memset(ones_bf[:], 1.0)

            mab_sb = []
            mabb_sb = []
            for s in range(2):
                m = constp.tile([P, NT * CK], FP8, name=f"mab_sb{s}")
                nc.sync.dma_start(out=m[:], in_=mabs[s][:])
                mab_sb.append(m)
                mb = constp.tile([P, NT * CK], BF16, name=f"mabb_sb{s}")
                nc.sync.dma_start(out=mb[:], in_=mabbs[s][:])
                mabb_sb.append(mb)

            # ---- persistent stashes ----
            zst = [stashp.tile([P, R], FP8, name=f"zst{s}") for s in range(2)]
            ssq = stashp.tile([P, 2 * NT], F32, name="ssq")
            cand8 = stashp.tile([P, NT * 8], F32, name="cand8")
            gselb = stashp.tile([P, NIDXP], FP8, name="gselb")

            # ---- DMA dispatch: z natural chunks (sync), zT (scalar queue) ----
            znat_bufs = []   # (t0, w, [tile per side])
            t0 = 0
            for gi, w in enumerate(CHN):
                row = []
                for s in range(2):
                    zg = zinp.tile([P, 18 * P], BF16, tag=f"zn{s}")
                    nc.sync.dma_start(
                        out=zg[:, : w * P], in_=zs[s][:, t0 * P : (t0 + w) * P]
                    )
                    row.append(zg)
                znat_bufs.append((t0, w, row))
                t0 += w
            t0 = 0
            for gi, w in enumerate(CHT):
                for s in range(2):
                    nc.scalar.dma_start(
                        out=zst[s][:, t0 * P : (t0 + w) * P],
                        in_=zts[s][:, t0 * P : (t0 + w) * P],
                    )
                t0 += w
            sel_sb = []
            e_sb = []
            for s in range(2):
                s0 = constp.tile([P, GPC], F32, name=f"sel_sb{s}a")
                s1 = constp.tile([P, GPC], F32, name=f"sel_sb{s}b")
                nc.sync.dma_start(out=s0[:], in_=sels[s][0:P, :])
                nc.sync.dma_start(out=s1[:], in_=sels[s][P : 2 * P, :])
                sel_sb.append((s0, s1))
                e = constp.tile([GPC, NIDXP], BF16, name=f"e_sb{s}")
                nc.sync.dma_start(out=e[:], in_=es[s][:])
                e_sb.append(e)

            # ---- P1 + S: seg matmuls, squares, row-sum matmuls ----
            pstag = [
                pstagp.tile([P, NT * CK], F32, name=f"pstag{s}") for s in range(2)
            ]
            pssq = pssqp.tile([P, 2 * NT], F32, name="pssq")
            for t0, w, row in znat_bufs:
                for s in range(2):
                    for k in range(w):
                        t = t0 + k
                        nc.tensor.matmul(
                            out=pstag[s][:, CK * t : CK * (t + 1)],
                            lhsT=row[s][:, k * P : (k + 1) * P],
                            rhs=mabb_sb[s][:, CK * t : CK * (t + 1)],
                            start=True,
                            stop=True,
                        )
                # squares: side 0 on Scalar, side 1 on DVE
                sq0 = sqp.tile([P, 18 * P], BF16, tag="sq0")
                nc.scalar.activation(
                    out=sq0[:, : w * P], in_=row[0][:, : w * P], func=AF.Square
                )
                sq1 = sqp.tile([P, 18 * P], BF16, tag="sq1")
                nc.vector.tensor_tensor(
                    out=sq1[:, : w * P], in0=row[1][:, : w * P],
                    in1=row[1][:, : w * P], op=AL.mult,
                )
                for s, sq in ((0, sq0), (1, sq1)):
                    for k in range(w):
                        t = t0 + k
                        nc.tensor.matmul(
                            out=pssq[:, s * NT + t : s * NT + t + 1],
                            lhsT=sq[:, k * P : (k + 1) * P],
                            rhs=ones_bf[:],
                            start=True,
                            stop=True,
                        )

            # ---- P2: transpose stag, scatter into local g ----
            pg = pgp.tile([GPC, 2 * P], F32, name="pg")
            for s in range(2):
                stag_sb = smallp.tile([P, NT * CK], F32, tag="stag", bufs=2)
                nc.scalar.copy(out=stag_sb[:], in_=pstag[s][:])
                rem = NT * CK - P  # 78
                stg_a = smallp.tile([P, P], F32, tag="stg", bufs=2)
                stg_b = smallp.tile([P, P], F32, tag="stg", bufs=2)
                pta = pztp.tile([P, P], F32, tag="pzt")
                nc.tensor.transpose(out=pta[:], in_=stag_sb[:, 0:P], identity=ident[:])
                nc.scalar.copy(out=stg_a[:], in_=pta[:])
                ptb = pztp.tile([P, P], F32, tag="pzt")
                nc.tensor.transpose(
                    out=ptb[:rem, :], in_=stag_sb[:, P : NT * CK], identity=ident[:]
                )
                nc.scalar.copy(out=stg_b[:rem, :], in_=ptb[:rem, :])
                nc.tensor.matmul(
                    out=pg[:, s * P : (s + 1) * P],
                    lhsT=sel_sb[s][0][:],
                    rhs=stg_a[:],
                    start=True,
                    stop=False,
                )
                nc.tensor.matmul(
                    out=pg[:, s * P : (s + 1) * P],
                    lhsT=sel_sb[s][1][:rem, :],
                    rhs=stg_b[:rem, :],
                    start=False,
                    stop=True,
                )

            # ---- P3: normalize g rows, cast to bf16 ----
            gf = smallp.tile([GPC, 2 * P], F32, tag="gf")
            nc.scalar.copy(out=gf[:], in_=pg[:])
            gnb = smallp.tile([GPC, 2 * P], BF16, tag="gnb")
            gsq = smallp.tile([GPC, 2 * P], F32, tag="gsq")
            gss = smallp.tile([GPC, 2], F32, tag="gss")
            for s in range(2):
                nc.scalar.activation(
                    out=gsq[:, s * P : (s + 1) * P],
                    in_=gf[:, s * P : (s + 1) * P],
                    func=AF.Square, accum_out=gss[:, s : s + 1],
                )
            gnr = smallp.tile([GPC, 2], F32, tag="gnrm", bufs=4)
            nc.scalar.sqrt(out=gnr[:], in_=gss[:])
            # row norms of z (ssq ready once P1 chunks done); same Sqrt table
            ssq_sb = smallp.tile([P, 2 * NT], F32, tag="rn", bufs=4)
            nc.vector.tensor_copy(out=ssq_sb[:], in_=pssq[:])
            rn = smallp.tile([P, 2 * NT], F32, tag="rn", bufs=4)
            nc.scalar.sqrt(out=rn[:], in_=ssq_sb[:])
            nc.vector.tensor_scalar(
                out=gnr[:], in0=gnr[:], scalar1=EPS, scalar2=None, op0=AL.max
            )
            ginv = smallp.tile([GPC, 2], F32, tag="gnrm", bufs=4)
            nc.vector.reciprocal(out=ginv[:], in_=gnr[:])
            for s in range(2):
                nc.scalar.mul(
                    out=gnb[:, s * P : (s + 1) * P],
                    in_=gf[:, s * P : (s + 1) * P],
                    mul=ginv[:, s : s + 1],
                )
            nc.vector.tensor_scalar(
                out=rn[:], in0=rn[:], scalar1=EPS, scalar2=None, op0=AL.max
            )
            inv = smallp.tile([P, 2 * NT], F32, tag="rn", bufs=4)
            nc.vector.reciprocal(out=inv[:], in_=rn[:])

            # ---- P5: gather candidate columns via one-hot matmuls ----
            H = NIDXP // 2  # 416
            for h in range(2):
                pgs = pztp.tile([P, H], F32, tag="pzt")
                for s in range(2):
                    nc.tensor.matmul(
                        out=pgs[:],
                        lhsT=gnb[:, s * P : (s + 1) * P],
                        rhs=e_sb[s][:, h * H : (h + 1) * H],
                        start=(s == 0),
                        stop=(s == 1),
                    )
                if h == 0:
                    nc.scalar.copy(out=gselb[:, h * H : (h + 1) * H], in_=pgs[:])
                else:
                    nc.vector.tensor_copy(
                        out=gselb[:, h * H : (h + 1) * H], in_=pgs[:]
                    )

            # ---- P6: per-tile matvecs, region-wise into 2 PSUM banks ----
            pcand = [
                pcandp.tile([P, HT * 8], F32, name=f"pcand{h}") for h in range(2)
            ]
            for t in range(NT):
                h, o = t // HT, (t % HT) * 8
                for s in range(2):
                    nc.tensor.matmul(
                        out=pcand[h][:, o + 4 * s : o + 4 * s + 4],
                        lhsT=zst[s][:, t * P : (t + 1) * P],
                        rhs=gselb[:, (t * 2 + s) * 4 : (t * 2 + s) * 4 + 4],
                        start=True,
                        stop=True,
                    )
            nc.vector.tensor_copy(out=cand8[:, : HT * 8], in_=pcand[0][:])
            nc.vector.tensor_copy(
                out=cand8[:, HT * 8 : NT * 8], in_=pcand[1][:, : (NT - HT) * 8]
            )

            # ---- P7: select (mask folded with 1/||z||), softplus, reduce ----
            # minv[p, 2t+j] = mab[p, 2t+j] * inv[p, t]
            inv2 = smallp.tile([P, NT * CK], F32, tag="inv2", bufs=2)
            minv = smallp.tile([P, NT * CK], F32, tag="minv", bufs=2)
            psel = smallp.tile([P, 2 * NT * CK], F32, tag="psel")
            spv = smallp.tile([P, 2 * NT * CK], F32, tag="spv")
            i2v = inv2[:].rearrange("p (t j) -> p t j", j=CK)
            cv = cand8[:].rearrange("p (t w) -> p t w", w=8)
            pv = psel[:].rearrange("p (s t j) -> p s t j", s=2, j=CK)
            for s in range(2):
                iv = inv[:, s * NT : (s + 1) * NT]
                for j in range(CK):
                    nc.vector.tensor_copy(out=i2v[:, :, j], in_=iv)
                nc.vector.tensor_tensor(
                    out=minv[:], in0=mabb_sb[s][:], in1=inv2[:], op=AL.mult
                )
                mv = minv[:].rearrange("p (t j) -> p t j", j=CK)
                ta4 = smallp.tile([P, NT * 4], F32, tag="ta4", bufs=2)
                t4 = ta4[:].rearrange("p (t q) -> p t q", q=4)
                nc.vector.tensor_tensor(
                    out=t4[:, :, 0:2], in0=mv[:], in1=cv[:, :, 4 * s : 4 * s + 2],
                    op=AL.mult,
                )
                nc.vector.tensor_tensor(
                    out=t4[:, :, 2:4], in0=mv[:], in1=cv[:, :, 4 * s + 2 : 4 * s + 4],
                    op=AL.mult,
                )
                # pairsum -> (pos, cross) interleaved
                nc.vector.tensor_tensor(
                    out=pv[:, s], in0=t4[:, :, 0::2], in1=t4[:, :, 1::2], op=AL.add
                )
            # q = ln(sigmoid(x)) = -softplus(-x), batched over both sides
            nc.scalar.activation(out=spv[:], in_=psel[:], func=AF.Sigmoid)
            nc.scalar.activation(out=spv[:], in_=spv[:], func=AF.Ln)
            sv = spv[:].rearrange("p (s t j) -> p s t j", s=2, j=CK)
            d2col = smallp.tile([P, 2], F32, tag="d2col")
            for s in range(2):
                # d = sp(-cross) - sp(-pos) = q_pos - q_cross
                dd = smallp.tile([P, NT], F32, tag="fin", bufs=4)
                nc.vector.tensor_tensor(
                    out=dd[:], in0=sv[:, s, :, 0], in1=sv[:, s, :, 1], op=AL.subtract
                )
                dsq = smallp.tile([P, NT], F32, tag="dsq", bufs=2)
                nc.scalar.activation(
                    out=dsq[:], in_=dd[:], func=AF.Square,
                    accum_out=d2col[:, s : s + 1],
                )

            pfin = pztp.tile([2, 1], F32, tag="pzt")
            nc.tensor.matmul(
                out=pfin[:], lhsT=d2col[:], rhs=ones_col[:], start=True, stop=True
            )
            osb = smallp.tile([2, 1], F32, tag="osb")
            nc.vector.tensor_copy(out=osb[:], in_=pfin[:])
            nc.sync.dma_start(out=out_part[:], in_=osb[:])

    with tile.TileContext(nc) as tc:
        _body(tc)
    if finalize:
        nc.finalize()
    return nc


def prep_inputs(z1, z2, batch_1, batch_2):
    """Graph-aligned shards + all index-derived input tensors (host-side)."""
    z1 = np.asarray(z1, dtype=np.float32)
    z2 = np.asarray(z2, dtype=np.float32)
    b1 = np.asarray(batch_1).astype(np.int64)
    b2 = np.asarray(batch_2).astype(np.int64)
    FP8H = ml_dtypes.float8_e4m3fn

    in_maps = []
    for c in range(NCORES):
        glo, ghi = c * GPC, (c + 1) * GPC
        m = {}
        idx_cols = np.zeros((NT, 2, 2 * CK), dtype=np.int64)
        for s, (z, b) in enumerate(((z1, b1), (z2, b2))):
            lo, hi = np.searchsorted(b, [glo, ghi])
            cnt = hi - lo
            assert cnt <= R, f"core {c} side {s}: {cnt} rows > {R}"
            zp = np.zeros((R, D), dtype=ml_dtypes.bfloat16)
            zp[:cnt] = z[lo:hi].astype(ml_dtypes.bfloat16)
            # tile-interleaved natural layout [128, R]
            m[f"z{s + 1}"] = np.ascontiguousarray(
                zp.reshape(NT, P, D).transpose(1, 2, 0).transpose(0, 2, 1).reshape(P, R)
            )
            m[f"zt{s + 1}"] = np.ascontiguousarray(zp.T.astype(FP8H))
            bt = np.full((R,), -1, dtype=np.int64)
            bt[:cnt] = b[lo:hi]
            btt = bt.reshape(NT, P)
            A = btt[:, 0].copy()
            A[A < 0] = glo
            vmax = btt.max(axis=1)
            assert (vmax - A <= CK - 1).all(), "tile spans >CK graphs"
            a = A - glo
            assert (a >= 0).all() and (a < GPC).all()
            mab = np.zeros((P, NT * CK), dtype=np.float32)
            for j in range(CK):
                mab[:, j::CK] = (btt == (A + j)[:, None]).T.astype(np.float32)
            m[f"mab{s + 1}"] = mab.astype(FP8H)
            m[f"mabb{s + 1}"] = mab.astype(ml_dtypes.bfloat16)
            sel = np.zeros((2 * P, GPC), dtype=np.float32)
            crow = np.arange(NT * CK)
            gid = np.repeat(a, CK) + np.tile(np.arange(CK), NT)
            ok = gid < GPC
            sel[crow[ok], gid[ok]] = 1.0
            m[f"sel{s + 1}"] = sel
            # candidate cols j = t*8 + side*4 + q: [own A, own A+1, other A, other A+1]
            idx_cols[:, s, :CK] = a[:, None] + np.arange(CK)
            idx_cols[:, s, CK:] = a[:, None] + np.arange(CK)
        # gather one-hots: E_s[w, j] = 1 iff col j sources side s, graph w
        for s in range(2):
            E = np.zeros((GPC, NIDXP), dtype=ml_dtypes.bfloat16)
            for t in range(NT):
                for side in range(2):
                    for q in range(2 * CK):
                        src = side if q < CK else 1 - side
                        if src != s:
                            continue
                        w = idx_cols[t, side, q]
                        if w < GPC:
                            E[w, t * 8 + side * 4 + q] = 1.0
            m[f"e{s + 1}"] = E
        in_maps.append(m)
    return in_maps


_NC_CACHE = {}


def _get_nc():
    if "nc" not in _NC_CACHE:
        _NC_CACHE["nc"] = build_nc()
    return _NC_CACHE["nc"]


def kernel(z1, z2, batch_1, batch_2):
    nc = _get_nc()
    in_maps = prep_inputs(z1, z2, batch_1, batch_2)
    res = run_bass_kernel_spmd(nc, in_maps, list(range(NCORES)))
    parts = np.stack([r["out_part"].reshape(2) for r in res.results])  # [8, 2]
    tot = parts.sum(axis=0)
    return np.float32(np.sqrt(tot[0]) + np.sqrt(tot[1]))
